# revision 1
# baseline (speedup 1.0000x reference)
"""Trainium2 Bass kernel for nn_DeconvLayer (causal IIR filter).

Math: the reference IIR v[i] = x[i] + sum_j w[j] v[i-1-j] (i >= F, else 0)
has a geometrically-decaying impulse response h (|h[128]| ~ 1e-13), so it
equals a 128-tap causal FIR applied to x with the first F columns zeroed:

    y[:, n] = sum_{k=0}^{127} h[k] * xz[:, n-k]

Implemented as block-Toeplitz matmuls on the TensorEngine:

    yT[c] = A0^T.T @ xT[c] + A1^T.T @ xT[c-1]

with A0[t,i] = h[i-t] (i >= t, incl. the exact 1.0 identity tap) and
A1[t,i] = h[128+i-t] (t > i), PSUM-accumulated.

Precision trick: x is split on the host into fp16 hi + lo (x = hi + lo to
~2^-22 relative), and A into fp16 Ahi + Alo.  Three fp16 matmul streams
(Ahi@hi + Ahi@lo + Alo@hi) give ~fp32 accuracy at full PE rate (fp16 runs
1 cycle/row vs 4 for fp32), with the same DMA traffic as fp32 input.

Layout trick: the host uploads x transposed AND 128-blocked as
[t, chunk, r] so time lands on the partition axis with no on-device
transposes and every DMA partition-line is one contiguous read.

Sharding: N = 131072 split into 8 column slabs of 16384 (+128-step halo
from the left neighbor), all B = 256 rows on every core.
"""

import os
import sys

import numpy as np

if "/opt/trn_rl_repo" not in sys.path:
    sys.path.insert(0, "/opt/trn_rl_repo")

B = 256
N = 131072
F = 8
K = 128          # FIR taps == block size
P = 128          # partitions / block size
NCORES = 8
CORE_COLS = N // NCORES       # 16384 time steps per core
NCHUNK = CORE_COLS // P       # 128 chunks per core
CPI = 16                      # chunks produced per iteration
NIT = NCHUNK // CPI           # 8 iterations per core
FREE = B                      # free dim per chunk (batch rows)
QG = CPI * FREE // 512        # 512-wide PSUM groups per iteration (8)

_CACHE = {}


def _impulse_response(w64):
    h = np.zeros(K, np.float64)
    h[0] = 1.0
    for n in range(1, K):
        acc = 0.0
        for j in range(min(F, n)):
            acc += w64[j] * h[n - 1 - j]
        h[n] = acc
    return h


def _toeplitz_mats(h):
    """A0[t, i] = h[i-t] for i >= t (incl. identity tap);
    A1[t, i] = h[128+i-t] for t > i.  Returned in float64."""
    a0 = np.zeros((P, P), np.float64)
    a1 = np.zeros((P, P), np.float64)
    for t in range(P):
        for i in range(P):
            if i >= t:
                a0[t, i] = h[i - t]
            elif t > i:
                a1[t, i] = h[K + i - t]
    return a0, a1


def _split16(a64):
    hi = a64.astype(np.float16)
    lo = (a64 - hi.astype(np.float64)).astype(np.float16)
    return hi, lo


def _build_nc():
    from contextlib import ExitStack

    import concourse.mybir as mybir
    import concourse.tile as tile
    from concourse import bacc

    f32 = mybir.dt.float32
    f16 = mybir.dt.float16

    nc = bacc.Bacc(
        "TRN2",
        target_bir_lowering=False,
        debug=False,
        enable_asserts=False,
        num_devices=NCORES,
    )
    # blocked transposed input: [t, chunk, r] flattened to [128, NCHUNK*FREE]
    W_IN = NCHUNK * FREE
    xhi_d = nc.dram_tensor("x_hi", [P, W_IN], f16, kind="ExternalInput")
    xlo_d = nc.dram_tensor("x_lo", [P, W_IN], f16, kind="ExternalInput")
    # halo: previous core's last 128 steps (zeros for core 0)
    hhi_d = nc.dram_tensor("h_hi", [P, FREE], f16, kind="ExternalInput")
    hlo_d = nc.dram_tensor("h_lo", [P, FREE], f16, kind="ExternalInput")
    a0hi_d = nc.dram_tensor("a0hi", [P, P], f16, kind="ExternalInput")
    a0lo_d = nc.dram_tensor("a0lo", [P, P], f16, kind="ExternalInput")
    a1hi_d = nc.dram_tensor("a1hi", [P, P], f16, kind="ExternalInput")
    a1lo_d = nc.dram_tensor("a1lo", [P, P], f16, kind="ExternalInput")
    # blocked transposed output [t, chunk, r]
    y_out = nc.dram_tensor("y_out", [P, NCHUNK * FREE], f32, kind="ExternalOutput")

    TW = CPI * FREE  # tile width (4096)

    with tile.TileContext(nc) as tc, ExitStack() as ctx:
        const = ctx.enter_context(tc.tile_pool(name="const", bufs=1))
        a_tiles = {}
        for name, d in [
            ("a0hi", a0hi_d),
            ("a0lo", a0lo_d),
            ("a1hi", a1hi_d),
            ("a1lo", a1lo_d),
        ]:
            t = const.tile([P, P], f16, tag=name)
            nc.scalar.dma_start(t[:], d[:, :])
            a_tiles[name] = t

        hpool = ctx.enter_context(tc.tile_pool(name="hi", bufs=4))
        lpool = ctx.enter_context(tc.tile_pool(name="lo", bufs=4))
        ypool = ctx.enter_context(tc.tile_pool(name="y", bufs=4))
        pspool = ctx.enter_context(tc.tile_pool(name="ps", bufs=8, space="PSUM"))

        # (stationary, moving, block-shift) per stream — stream-major order
        STREAMS = [
            ("a0hi", "hi", 0),
            ("a0hi", "lo", 0),
            ("a0lo", "hi", 0),
            ("a1hi", "hi", 1),
            ("a1hi", "lo", 1),
            ("a1lo", "hi", 1),
        ]

        prev = None
        for it in range(NIT):
            u0 = it * TW
            # tiles carry a leading halo chunk: [halo(256) | 16 chunks(4096)]
            hi = hpool.tile([P, FREE + TW], f16)
            lo = lpool.tile([P, FREE + TW], f16)
            if it == 0:
                nc.sync.dma_start(hi[:, :FREE], hhi_d[:, :])
                nc.sync.dma_start(lo[:, :FREE], hlo_d[:, :])
            else:
                # halo = previous tile's last chunk, copied within SBUF on the
                # otherwise-idle GpSimd engine
                nc.gpsimd.tensor_copy(hi[:, :FREE], prev[0][:, TW : TW + FREE])
                nc.gpsimd.tensor_copy(lo[:, :FREE], prev[1][:, TW : TW + FREE])
            # two half-loads for finer dependency granularity
            H = TW // 2
            nc.sync.dma_start(hi[:, FREE : FREE + H], xhi_d[:, u0 : u0 + H])
            nc.sync.dma_start(hi[:, FREE + H :], xhi_d[:, u0 + H : u0 + TW])
            nc.sync.dma_start(lo[:, FREE : FREE + H], xlo_d[:, u0 : u0 + H])
            nc.sync.dma_start(lo[:, FREE + H :], xlo_d[:, u0 + H : u0 + TW])
            xin = {"hi": hi, "lo": lo}
            prev = (hi, lo)

            ybuf = ypool.tile([P, TW], f32)
            pss = [
                pspool.tile([P, 512], f32, name=f"ps_{it}_{q}", tag="ps")
                for q in range(QG)
            ]
            for s, (a_name, x_name, shift) in enumerate(STREAMS):
                a_t = a_tiles[a_name]
                start = s == 0
                stop = s == len(STREAMS) - 1
                for q in range(QG):
                    off = (1 - shift) * FREE + q * 512
                    nc.tensor.matmul(
                        pss[q][:],
                        a_t[:],
                        xin[x_name][:, off : off + 512],
                        start=start,
                        stop=stop,
                    )
            for q in range(QG):
                if q % 2 == 0:
                    nc.vector.tensor_copy(ybuf[:, q * 512 : (q + 1) * 512], pss[q][:])
                else:
                    nc.scalar.copy(ybuf[:, q * 512 : (q + 1) * 512], pss[q][:])

            # output on the second HWDGE ring (ACT)
            nc.scalar.dma_start(y_out[:, u0 : u0 + TW], ybuf[:])
    nc.compile()
    return nc


def _get_nc():
    if "nc" not in _CACHE:
        _CACHE["nc"] = _build_nc()
    return _CACHE["nc"]


LAST_RESULTS = None


def kernel(x, w=None, _trace=False, **_ignored):
    global LAST_RESULTS
    from concourse.bass_utils import run_bass_kernel_spmd

    x = np.asarray(x, dtype=np.float32)
    assert x.shape == (B, N)
    if w is None:
        import jax
        import jax.numpy as jnp

        key = jax.random.key(0)
        _, k2 = jax.random.split(key)
        w = np.asarray(jax.random.normal(k2, (F,), dtype=jnp.float32) * 0.05)
    w = np.asarray(w, dtype=np.float32)

    h = _impulse_response(w.astype(np.float64))
    a0, a1 = _toeplitz_mats(h)
    a0hi, a0lo = _split16(a0)
    a1hi, a1lo = _split16(a1)

    # transposed, 128-blocked input: [t, chunk, r]
    xt = np.array(x.T)  # [N, B]
    xt[:F] = 0.0  # v[i] = 0 for i < F
    xb = np.ascontiguousarray(
        xt.reshape(NCORES * NCHUNK, P, B).transpose(1, 0, 2)
    )  # [128, 1024, 256] fp32
    xb_hi = xb.astype(np.float16)
    xb_lo = (xb - xb_hi.astype(np.float32)).astype(np.float16)
    zhalo = np.zeros((P, B), np.float16)

    in_maps = []
    for c in range(NCORES):
        lo_c = c * NCHUNK
        sl = np.s_[:, lo_c : lo_c + NCHUNK, :]
        in_maps.append(
            {
                "x_hi": np.ascontiguousarray(xb_hi[sl]).reshape(P, -1),
                "x_lo": np.ascontiguousarray(xb_lo[sl]).reshape(P, -1),
                "h_hi": zhalo if c == 0 else np.ascontiguousarray(xb_hi[:, lo_c - 1, :]),
                "h_lo": zhalo if c == 0 else np.ascontiguousarray(xb_lo[:, lo_c - 1, :]),
                "a0hi": a0hi,
                "a0lo": a0lo,
                "a1hi": a1hi,
                "a1lo": a1lo,
            }
        )

    nc = _get_nc()
    res = run_bass_kernel_spmd(
        nc, in_maps, core_ids=list(range(NCORES)), trace=_trace
    )
    LAST_RESULTS = res
    # reassemble: per core [128, NCHUNK, FREE] -> [NCHUNK*P, FREE]
    parts = []
    for r in res.results:
        yb = r["y_out"].reshape(P, NCHUNK, B).transpose(1, 0, 2)  # [chunk, t, r]
        parts.append(yb.reshape(CORE_COLS, B))
    yt = np.concatenate(parts, axis=0)  # [N, B]
    return np.ascontiguousarray(yt.T)


if __name__ == "__main__":
    rng = np.random.default_rng(0)
    x = rng.standard_normal((B, N), dtype=np.float32)
    w = (rng.standard_normal(F) * 0.05).astype(np.float32)
    y = kernel(x, w)
    print("kernel ran, y shape:", y.shape)



# revision 2
# speedup vs baseline: 1.8561x; 1.8561x over previous
"""Trainium2 Bass kernel for nn_DeconvLayer (causal IIR filter).

Math: the reference IIR v[i] = x[i] + sum_j w[j] v[i-1-j] (i >= F, else 0)
has a geometrically-decaying impulse response h (|h[128]| ~ 1e-13), so it
equals a 128-tap causal FIR applied to x with the first F columns zeroed:

    y[:, n] = sum_{k=0}^{127} h[k] * xz[:, n-k]

Implemented as block-Toeplitz matmuls on the TensorEngine:

    yT[c] = A0^T.T @ xT[c] + A1^T.T @ xT[c-1]

with A0[t,i] = h[i-t] (i >= t, incl. the exact 1.0 identity tap) and
A1[t,i] = h[128+i-t] (t > i), PSUM-accumulated.

Precision: the correctness gate is rel_err < 2e-2; a single fp16 stream
(x, A, and y all fp16, fp32 PSUM accumulation) lands at ~3e-4, so no
hi/lo splitting is needed.  This halves HBM traffic vs fp32 output and
cuts the matmul stream count to 2 (a0@x + a1@x), both of which matter:
the kernel is DMA-bound at ~358 GB/s/core.

Layout trick: the host uploads x transposed AND 128-blocked as
[t, chunk, r] so time lands on the partition axis with no on-device
transposes and every DMA partition-line is one contiguous read.

Sharding: N = 131072 split into 8 column slabs of 16384 (+128-step halo
from the left neighbor), all B = 256 rows on every core.
"""

import os
import sys

import numpy as np

if "/opt/trn_rl_repo" not in sys.path:
    sys.path.insert(0, "/opt/trn_rl_repo")

B = 256
N = 131072
F = 8
K = 128          # FIR taps == block size
P = 128          # partitions / block size
NCORES = 8
CORE_COLS = N // NCORES       # 16384 time steps per core
NCHUNK = CORE_COLS // P       # 128 chunks per core
CPI = 16                      # chunks produced per iteration
NIT = NCHUNK // CPI           # 8 iterations per core
FREE = B                      # free dim per chunk (batch rows)
QG = CPI * FREE // 512        # 512-wide PSUM groups per iteration (8)

_CACHE = {}


def _impulse_response(w64):
    h = np.zeros(K, np.float64)
    h[0] = 1.0
    for n in range(1, K):
        acc = 0.0
        for j in range(min(F, n)):
            acc += w64[j] * h[n - 1 - j]
        h[n] = acc
    return h


def _toeplitz_mats(h):
    """A0[t, i] = h[i-t] for i >= t (incl. identity tap);
    A1[t, i] = h[128+i-t] for t > i.  Returned in float64."""
    a0 = np.zeros((P, P), np.float64)
    a1 = np.zeros((P, P), np.float64)
    for t in range(P):
        for i in range(P):
            if i >= t:
                a0[t, i] = h[i - t]
            elif t > i:
                a1[t, i] = h[K + i - t]
    return a0, a1


def _build_nc():
    from contextlib import ExitStack

    import concourse.mybir as mybir
    import concourse.tile as tile
    from concourse import bacc

    f16 = mybir.dt.float16

    nc = bacc.Bacc(
        "TRN2",
        target_bir_lowering=False,
        debug=False,
        enable_asserts=False,
        num_devices=NCORES,
    )
    # blocked transposed input: [t, chunk, r] flattened to [128, NCHUNK*FREE]
    W_IN = NCHUNK * FREE
    x_d = nc.dram_tensor("x_in", [P, W_IN], f16, kind="ExternalInput")
    # halo: previous core's last 128 steps (zeros for core 0)
    h_d = nc.dram_tensor("h_in", [P, FREE], f16, kind="ExternalInput")
    a0_d = nc.dram_tensor("a0", [P, P], f16, kind="ExternalInput")
    a1_d = nc.dram_tensor("a1", [P, P], f16, kind="ExternalInput")
    # blocked transposed output [t, chunk, r]
    y_out = nc.dram_tensor("y_out", [P, NCHUNK * FREE], f16, kind="ExternalOutput")

    TW = CPI * FREE  # tile width (4096)

    with tile.TileContext(nc) as tc, ExitStack() as ctx:
        const = ctx.enter_context(tc.tile_pool(name="const", bufs=1))
        a_tiles = {}
        for name, d in [("a0", a0_d), ("a1", a1_d)]:
            t = const.tile([P, P], f16, tag=name)
            nc.scalar.dma_start(t[:], d[:, :])
            a_tiles[name] = t

        xpool = ctx.enter_context(tc.tile_pool(name="x", bufs=4))
        ypool = ctx.enter_context(tc.tile_pool(name="y", bufs=4))
        pspool = ctx.enter_context(tc.tile_pool(name="ps", bufs=8, space="PSUM"))

        # (stationary, block-shift) per stream — stream-major order
        STREAMS = [("a0", 0), ("a1", 1)]

        prev = None
        for it in range(NIT):
            u0 = it * TW
            # tiles carry a leading halo chunk: [halo(256) | 16 chunks(4096)]
            xt = xpool.tile([P, FREE + TW], f16)
            if it == 0:
                nc.sync.dma_start(xt[:, :FREE], h_d[:, :])
            else:
                # halo = previous tile's last chunk, copied within SBUF on the
                # otherwise-idle GpSimd engine
                nc.gpsimd.tensor_copy(xt[:, :FREE], prev[:, TW : TW + FREE])
            # two half-loads for finer dependency granularity
            H = TW // 2
            nc.sync.dma_start(xt[:, FREE : FREE + H], x_d[:, u0 : u0 + H])
            nc.sync.dma_start(xt[:, FREE + H :], x_d[:, u0 + H : u0 + TW])
            prev = xt

            ybuf = ypool.tile([P, TW], f16)
            pss = [
                pspool.tile([P, 512], mybir.dt.float32, name=f"ps_{it}_{q}", tag="ps")
                for q in range(QG)
            ]
            for s, (a_name, shift) in enumerate(STREAMS):
                a_t = a_tiles[a_name]
                start = s == 0
                stop = s == len(STREAMS) - 1
                for q in range(QG):
                    off = (1 - shift) * FREE + q * 512
                    nc.tensor.matmul(
                        pss[q][:],
                        a_t[:],
                        xt[:, off : off + 512],
                        start=start,
                        stop=stop,
                    )
            for q in range(QG):
                if q % 2 == 0:
                    nc.vector.tensor_copy(ybuf[:, q * 512 : (q + 1) * 512], pss[q][:])
                else:
                    nc.scalar.copy(ybuf[:, q * 512 : (q + 1) * 512], pss[q][:])

            # output on the second HWDGE ring (ACT)
            nc.scalar.dma_start(y_out[:, u0 : u0 + TW], ybuf[:])
    nc.compile()
    return nc


def _get_nc():
    if "nc" not in _CACHE:
        _CACHE["nc"] = _build_nc()
    return _CACHE["nc"]


LAST_RESULTS = None


def kernel(x, w=None, _trace=False, **_ignored):
    global LAST_RESULTS
    from concourse.bass_utils import run_bass_kernel_spmd

    x = np.asarray(x, dtype=np.float32)
    assert x.shape == (B, N)
    if w is None:
        import jax
        import jax.numpy as jnp

        key = jax.random.key(0)
        _, k2 = jax.random.split(key)
        w = np.asarray(jax.random.normal(k2, (F,), dtype=jnp.float32) * 0.05)
    w = np.asarray(w, dtype=np.float32)

    h = _impulse_response(w.astype(np.float64))
    a0, a1 = _toeplitz_mats(h)
    a0h = a0.astype(np.float16)
    a1h = a1.astype(np.float16)

    # transposed, 128-blocked input: [t, chunk, r]
    xt = np.array(x.T)  # [N, B]
    xt[:F] = 0.0  # v[i] = 0 for i < F
    xb = np.ascontiguousarray(
        xt.reshape(NCORES * NCHUNK, P, B).transpose(1, 0, 2)
    ).astype(np.float16)  # [128, 1024, 256]
    zhalo = np.zeros((P, B), np.float16)

    in_maps = []
    for c in range(NCORES):
        lo_c = c * NCHUNK
        sl = np.s_[:, lo_c : lo_c + NCHUNK, :]
        in_maps.append(
            {
                "x_in": np.ascontiguousarray(xb[sl]).reshape(P, -1),
                "h_in": zhalo if c == 0 else np.ascontiguousarray(xb[:, lo_c - 1, :]),
                "a0": a0h,
                "a1": a1h,
            }
        )

    nc = _get_nc()
    res = run_bass_kernel_spmd(
        nc, in_maps, core_ids=list(range(NCORES)), trace=_trace
    )
    LAST_RESULTS = res
    # reassemble: per core [128, NCHUNK, FREE] -> [NCHUNK*P, FREE]
    parts = []
    for r in res.results:
        yb = r["y_out"].astype(np.float32).reshape(P, NCHUNK, B).transpose(1, 0, 2)
        parts.append(yb.reshape(CORE_COLS, B))
    yt = np.concatenate(parts, axis=0)  # [N, B]
    return np.ascontiguousarray(yt.T)


if __name__ == "__main__":
    rng = np.random.default_rng(0)
    x = rng.standard_normal((B, N), dtype=np.float32)
    w = (rng.standard_normal(F) * 0.05).astype(np.float32)
    y = kernel(x, w)
    print("kernel ran, y shape:", y.shape)


# revision 4
# speedup vs baseline: 1.8772x; 1.0114x over previous
"""Trainium2 Bass kernel for nn_DeconvLayer (causal IIR filter).

Math: the reference IIR v[i] = x[i] + sum_j w[j] v[i-1-j] (i >= F, else 0)
has a geometrically-decaying impulse response h (|h[128]| ~ 1e-13), so it
equals a 128-tap causal FIR applied to x with the first F columns zeroed:

    y[:, n] = sum_{k=0}^{127} h[k] * xz[:, n-k]

Implemented as block-Toeplitz matmuls on the TensorEngine:

    yT[c] = A0^T.T @ xT[c] + A1^T.T @ xT[c-1]

with A0[t,i] = h[i-t] (i >= t, incl. the exact 1.0 identity tap) and
A1[t,i] = h[128+i-t] (t > i), PSUM-accumulated.

Precision: the correctness gate is rel_err < 2e-2; a single fp16 stream
(x, A, and y all fp16, fp32 PSUM accumulation) lands at ~3e-4, so no
hi/lo splitting is needed.  This halves HBM traffic vs fp32 output and
cuts the matmul stream count to 2 (a0@x + a1@x), both of which matter:
the kernel is DMA-bound at ~358 GB/s/core.

Layout trick: the host uploads x transposed AND 128-blocked as
[t, chunk, r] so time lands on the partition axis with no on-device
transposes and every DMA partition-line is one contiguous read.

Sharding: N = 131072 split into 8 column slabs of 16384 (+128-step halo
from the left neighbor), all B = 256 rows on every core.
"""

import os
import sys

import numpy as np

if "/opt/trn_rl_repo" not in sys.path:
    sys.path.insert(0, "/opt/trn_rl_repo")

B = 256
N = 131072
F = 8
K = 128          # FIR taps == block size
P = 128          # partitions / block size
NCORES = 8
CORE_COLS = N // NCORES       # 16384 time steps per core
NCHUNK = CORE_COLS // P       # 128 chunks per core
CPI = 8                       # chunks produced per iteration
NIT = NCHUNK // CPI           # 16 iterations per core
FREE = B                      # free dim per chunk (batch rows)
QG = CPI * FREE // 512        # 512-wide PSUM groups per iteration (4)
NPAIR = QG // 2               # 1024-wide PSUM pair-tiles per iteration (2)

_CACHE = {}


def _impulse_response(w64):
    h = np.zeros(K, np.float64)
    h[0] = 1.0
    for n in range(1, K):
        acc = 0.0
        for j in range(min(F, n)):
            acc += w64[j] * h[n - 1 - j]
        h[n] = acc
    return h


def _toeplitz_mats(h):
    """A0[t, i] = h[i-t] for i >= t (incl. identity tap);
    A1[t, i] = h[128+i-t] for t > i.  Returned in float64."""
    a0 = np.zeros((P, P), np.float64)
    a1 = np.zeros((P, P), np.float64)
    for t in range(P):
        for i in range(P):
            if i >= t:
                a0[t, i] = h[i - t]
            elif t > i:
                a1[t, i] = h[K + i - t]
    return a0, a1


def _build_nc():
    from contextlib import ExitStack

    import concourse.mybir as mybir
    import concourse.tile as tile
    from concourse import bacc

    f16 = mybir.dt.float16

    nc = bacc.Bacc(
        "TRN2",
        target_bir_lowering=False,
        debug=False,
        enable_asserts=False,
        num_devices=NCORES,
    )
    # blocked transposed input: [t, chunk, r] flattened to [128, NCHUNK*FREE]
    W_IN = NCHUNK * FREE
    x_d = nc.dram_tensor("x_in", [P, W_IN], f16, kind="ExternalInput")
    # halo: previous core's last 128 steps (zeros for core 0)
    h_d = nc.dram_tensor("h_in", [P, FREE], f16, kind="ExternalInput")
    a0_d = nc.dram_tensor("a0", [P, P], f16, kind="ExternalInput")
    a1_d = nc.dram_tensor("a1", [P, P], f16, kind="ExternalInput")
    # blocked transposed output [t, chunk, r]
    y_out = nc.dram_tensor("y_out", [P, NCHUNK * FREE], f16, kind="ExternalOutput")

    TW = CPI * FREE  # tile width (4096)

    with tile.TileContext(nc) as tc, ExitStack() as ctx:
        const = ctx.enter_context(tc.tile_pool(name="const", bufs=1))
        a_tiles = {}
        for name, d in [("a0", a0_d), ("a1", a1_d)]:
            t = const.tile([P, P], f16, tag=name)
            nc.scalar.dma_start(t[:], d[:, :])
            a_tiles[name] = t

        xpool = ctx.enter_context(tc.tile_pool(name="x", bufs=4))
        ypool = ctx.enter_context(tc.tile_pool(name="y", bufs=4))
        # 2 pair-tiles (2 PSUM banks each) per iteration, double-buffered
        # across iterations so matmuls never wait on the previous drain
        pspool = ctx.enter_context(tc.tile_pool(name="ps", bufs=4, space="PSUM"))

        prev = None
        for it in range(NIT):
            u0 = it * TW
            # tiles carry a leading halo chunk: [halo(256) | 8 chunks(2048)]
            xt = xpool.tile([P, FREE + TW], f16)
            if it == 0:
                nc.sync.dma_start(xt[:, :FREE], h_d[:, :])
            else:
                # halo = previous tile's last chunk, copied within SBUF on the
                # otherwise-idle GpSimd engine
                nc.gpsimd.tensor_copy(xt[:, :FREE], prev[:, TW : TW + FREE])
            nc.sync.dma_start(xt[:, FREE:], x_d[:, u0 : u0 + TW])
            prev = xt

            ybuf = ypool.tile([P, TW], f16)
            for p in range(NPAIR):
                ps = pspool.tile(
                    [P, 1024], mybir.dt.float32, name=f"ps_{it}_{p}", tag="ps"
                )
                # pair-major, stream-inner order: both a0 matmuls, then both
                # a1 (each 512 sub-region gets a0 start / a1 stop), so the
                # pair completes early and its 1024-wide cast overlaps the
                # next pair's matmuls
                for s, (a_name, shift) in enumerate([("a0", 0), ("a1", 1)]):
                    a_t = a_tiles[a_name]
                    for h in range(2):
                        off = (1 - shift) * FREE + p * 1024 + h * 512
                        nc.tensor.matmul(
                            ps[:, h * 512 : (h + 1) * 512],
                            a_t[:],
                            xt[:, off : off + 512],
                            start=s == 0,
                            stop=s == 1,
                        )
                # PSUM->SBUF drain: vector takes even pairs, scalar odd
                dst = ybuf[:, p * 1024 : (p + 1) * 1024]
                if p % 2 == 0:
                    nc.vector.tensor_copy(dst, ps[:])
                else:
                    nc.scalar.copy(dst, ps[:])

            # output on the second HWDGE ring (ACT)
            nc.scalar.dma_start(y_out[:, u0 : u0 + TW], ybuf[:])
    nc.compile()
    return nc


def _get_nc():
    if "nc" not in _CACHE:
        _CACHE["nc"] = _build_nc()
    return _CACHE["nc"]


LAST_RESULTS = None


def kernel(x, w=None, _trace=False, **_ignored):
    global LAST_RESULTS
    from concourse.bass_utils import run_bass_kernel_spmd

    x = np.asarray(x, dtype=np.float32)
    assert x.shape == (B, N)
    if w is None:
        import jax
        import jax.numpy as jnp

        key = jax.random.key(0)
        _, k2 = jax.random.split(key)
        w = np.asarray(jax.random.normal(k2, (F,), dtype=jnp.float32) * 0.05)
    w = np.asarray(w, dtype=np.float32)

    h = _impulse_response(w.astype(np.float64))
    a0, a1 = _toeplitz_mats(h)
    a0h = a0.astype(np.float16)
    a1h = a1.astype(np.float16)

    # transposed, 128-blocked input: [t, chunk, r]
    xt = np.array(x.T)  # [N, B]
    xt[:F] = 0.0  # v[i] = 0 for i < F
    xb = np.ascontiguousarray(
        xt.reshape(NCORES * NCHUNK, P, B).transpose(1, 0, 2)
    ).astype(np.float16)  # [128, 1024, 256]
    zhalo = np.zeros((P, B), np.float16)

    in_maps = []
    for c in range(NCORES):
        lo_c = c * NCHUNK
        sl = np.s_[:, lo_c : lo_c + NCHUNK, :]
        in_maps.append(
            {
                "x_in": np.ascontiguousarray(xb[sl]).reshape(P, -1),
                "h_in": zhalo if c == 0 else np.ascontiguousarray(xb[:, lo_c - 1, :]),
                "a0": a0h,
                "a1": a1h,
            }
        )

    nc = _get_nc()
    res = run_bass_kernel_spmd(
        nc, in_maps, core_ids=list(range(NCORES)), trace=_trace
    )
    LAST_RESULTS = res
    # reassemble: per core [128, NCHUNK, FREE] -> [NCHUNK*P, FREE]
    parts = []
    for r in res.results:
        yb = r["y_out"].astype(np.float32).reshape(P, NCHUNK, B).transpose(1, 0, 2)
        parts.append(yb.reshape(CORE_COLS, B))
    yt = np.concatenate(parts, axis=0)  # [N, B]
    return np.ascontiguousarray(yt.T)


if __name__ == "__main__":
    rng = np.random.default_rng(0)
    x = rng.standard_normal((B, N), dtype=np.float32)
    w = (rng.standard_normal(F) * 0.05).astype(np.float32)
    y = kernel(x, w)
    print("kernel ran, y shape:", y.shape)


# revision 5
# speedup vs baseline: 2.3811x; 1.2685x over previous
"""Trainium2 Bass kernel for nn_DeconvLayer (causal IIR filter).

Math: the reference IIR v[i] = x[i] + sum_j w[j] v[i-1-j] (i >= F, else 0)
has a geometrically-decaying impulse response h (|h[128]| ~ 1e-13), so it
equals a 128-tap causal FIR applied to x with the first F columns zeroed.

The kernel is HBM-bound (358 GB/s/core), so the entire design minimizes
bytes moved.  Device computes only the small CORRECTION

    c = y - x = (h - delta) * xz        (xz = x with first F cols zeroed)

as block-Toeplitz matmuls  cT[b] = A0'^T.T @ xT[b] + A1^T.T @ xT[b-1]
with A0' = A0 - I (identity tap dropped) — and the host adds x back in
fp32.  Since ||c|| ~ 0.18 ||y||, both the input x and the output c can be
stored in fp8 e4m3 (~2.7% RMS rounding) while keeping the end-to-end
relative error ~1e-2, under the 2e-2 gate:

    in 4.2 MB + out 4.2 MB per core  ->  ~24 us DMA floor
    (vs 33.6 MB / ~94 us for the fp32-precise variant)

fp8 matmuls run at bf16 speed on the PE (no perf mode needed); PSUM
accumulates in fp32; the PSUM->SBUF drain casts fp32 -> e4m3 on the
Vector/Scalar engines (different banks in parallel).

Layout trick: the host uploads x transposed AND 128-blocked as
[t, chunk, r] so time lands on the partition axis with no on-device
transposes and every DMA partition-line is one contiguous read.

Sharding: N = 131072 split into 8 column slabs of 16384 (+128-step halo
from the left neighbor), all B = 256 rows on every core.
"""

import os
import sys

import numpy as np

if "/opt/trn_rl_repo" not in sys.path:
    sys.path.insert(0, "/opt/trn_rl_repo")

B = 256
N = 131072
F = 8
K = 128          # FIR taps == block size
P = 128          # partitions / block size
NCORES = 8
CORE_COLS = N // NCORES       # 16384 time steps per core
NCHUNK = CORE_COLS // P       # 128 chunks per core
CPI = 8                       # chunks produced per iteration
NIT = NCHUNK // CPI           # 16 iterations per core
FREE = B                      # free dim per chunk (batch rows)
QG = CPI * FREE // 512        # 512-wide PSUM groups per iteration (4)
NPAIR = QG // 2               # 1024-wide PSUM pair-tiles per iteration (2)

_CACHE = {}


def _impulse_response(w64):
    h = np.zeros(K, np.float64)
    h[0] = 1.0
    for n in range(1, K):
        acc = 0.0
        for j in range(min(F, n)):
            acc += w64[j] * h[n - 1 - j]
        h[n] = acc
    return h


def _toeplitz_mats(h):
    """A0[t, i] = h[i-t] for i > t (identity tap EXCLUDED -> correction);
    A1[t, i] = h[128+i-t] for t > i.  Returned in float64."""
    a0 = np.zeros((P, P), np.float64)
    a1 = np.zeros((P, P), np.float64)
    for t in range(P):
        for i in range(P):
            if i > t:
                a0[t, i] = h[i - t]
            elif t > i:
                a1[t, i] = h[K + i - t]
    return a0, a1


def _build_nc():
    from contextlib import ExitStack

    import concourse.mybir as mybir
    import concourse.tile as tile
    from concourse import bacc

    f8 = mybir.dt.float8e4

    nc = bacc.Bacc(
        "TRN2",
        target_bir_lowering=False,
        debug=False,
        enable_asserts=False,
        num_devices=NCORES,
    )
    # blocked transposed input: [t, chunk, r] flattened to [128, NCHUNK*FREE]
    W_IN = NCHUNK * FREE
    x_d = nc.dram_tensor("x_in", [P, W_IN], f8, kind="ExternalInput")
    # halo: previous core's last 128 steps (zeros for core 0)
    h_d = nc.dram_tensor("h_in", [P, FREE], f8, kind="ExternalInput")
    a0_d = nc.dram_tensor("a0", [P, P], f8, kind="ExternalInput")
    a1_d = nc.dram_tensor("a1", [P, P], f8, kind="ExternalInput")
    # blocked transposed correction output [t, chunk, r]
    y_out = nc.dram_tensor("y_out", [P, NCHUNK * FREE], f8, kind="ExternalOutput")

    TW = CPI * FREE  # tile width (2048)

    with tile.TileContext(nc) as tc, ExitStack() as ctx:
        const = ctx.enter_context(tc.tile_pool(name="const", bufs=1))
        a_tiles = {}
        for name, d in [("a0", a0_d), ("a1", a1_d)]:
            t = const.tile([P, P], f8, tag=name)
            nc.scalar.dma_start(t[:], d[:, :])
            a_tiles[name] = t

        xpool = ctx.enter_context(tc.tile_pool(name="x", bufs=4))
        ypool = ctx.enter_context(tc.tile_pool(name="y", bufs=4))
        # 2 pair-tiles (2 PSUM banks each) per iteration, double-buffered
        # across iterations so matmuls never wait on the previous drain
        pspool = ctx.enter_context(tc.tile_pool(name="ps", bufs=4, space="PSUM"))

        prev = None
        for it in range(NIT):
            u0 = it * TW
            # tiles carry a leading halo chunk: [halo(256) | 8 chunks(2048)]
            xt = xpool.tile([P, FREE + TW], f8)
            if it == 0:
                nc.sync.dma_start(xt[:, :FREE], h_d[:, :])
            else:
                # halo = previous tile's last chunk, copied within SBUF on the
                # otherwise-idle GpSimd engine
                nc.gpsimd.tensor_copy(xt[:, :FREE], prev[:, TW : TW + FREE])
            nc.sync.dma_start(xt[:, FREE:], x_d[:, u0 : u0 + TW])
            prev = xt

            ybuf = ypool.tile([P, TW], f8)
            for p in range(NPAIR):
                ps = pspool.tile(
                    [P, 1024], mybir.dt.float32, name=f"ps_{it}_{p}", tag="ps"
                )
                # pair-major, stream-inner order: both a0 matmuls, then both
                # a1 (each 512 sub-region gets a0 start / a1 stop), so the
                # pair completes early and its 1024-wide cast overlaps the
                # next pair's matmuls
                for s, (a_name, shift) in enumerate([("a0", 0), ("a1", 1)]):
                    a_t = a_tiles[a_name]
                    for h in range(2):
                        off = (1 - shift) * FREE + p * 1024 + h * 512
                        nc.tensor.matmul(
                            ps[:, h * 512 : (h + 1) * 512],
                            a_t[:],
                            xt[:, off : off + 512],
                            start=s == 0,
                            stop=s == 1,
                        )
                # PSUM->SBUF drain with fp32 -> e4m3 cast: vector takes even
                # pairs, scalar odd (parallel access to different banks)
                dst = ybuf[:, p * 1024 : (p + 1) * 1024]
                if p % 2 == 0:
                    nc.vector.tensor_copy(dst, ps[:])
                else:
                    nc.scalar.copy(dst, ps[:])

            # output on the second HWDGE ring (ACT)
            nc.scalar.dma_start(y_out[:, u0 : u0 + TW], ybuf[:])
    nc.compile()
    return nc


def _get_nc():
    if "nc" not in _CACHE:
        _CACHE["nc"] = _build_nc()
    return _CACHE["nc"]


LAST_RESULTS = None


def kernel(x, w=None, _trace=False, **_ignored):
    global LAST_RESULTS
    import ml_dtypes
    from concourse.bass_utils import run_bass_kernel_spmd

    f8 = ml_dtypes.float8_e4m3

    x = np.asarray(x, dtype=np.float32)
    assert x.shape == (B, N)
    if w is None:
        import jax
        import jax.numpy as jnp

        key = jax.random.key(0)
        _, k2 = jax.random.split(key)
        w = np.asarray(jax.random.normal(k2, (F,), dtype=jnp.float32) * 0.05)
    w = np.asarray(w, dtype=np.float32)

    h = _impulse_response(w.astype(np.float64))
    a0, a1 = _toeplitz_mats(h)
    a0q = a0.astype(f8)
    a1q = a1.astype(f8)

    # transposed, 128-blocked input: [t, chunk, r]
    xt = np.array(x.T)  # [N, B]
    xt[:F] = 0.0  # v[i] = 0 for i < F
    xb = np.ascontiguousarray(
        xt.reshape(NCORES * NCHUNK, P, B).transpose(1, 0, 2)
    ).astype(f8)  # [128, 1024, 256]
    zhalo = np.zeros((P, B), f8)

    in_maps = []
    for c in range(NCORES):
        lo_c = c * NCHUNK
        sl = np.s_[:, lo_c : lo_c + NCHUNK, :]
        in_maps.append(
            {
                "x_in": np.ascontiguousarray(xb[sl]).reshape(P, -1),
                "h_in": zhalo if c == 0 else np.ascontiguousarray(xb[:, lo_c - 1, :]),
                "a0": a0q,
                "a1": a1q,
            }
        )

    nc = _get_nc()
    res = run_bass_kernel_spmd(
        nc, in_maps, core_ids=list(range(NCORES)), trace=_trace
    )
    LAST_RESULTS = res
    # reassemble: per core [128, NCHUNK, FREE] -> [NCHUNK*P, FREE]
    parts = []
    for r in res.results:
        cb = (
            np.asarray(r["y_out"])
            .astype(np.float32)
            .reshape(P, NCHUNK, B)
            .transpose(1, 0, 2)
        )
        parts.append(cb.reshape(CORE_COLS, B))
    ct = np.concatenate(parts, axis=0)  # correction, [N, B]
    y = x + np.ascontiguousarray(ct.T)  # add identity tap back in fp32
    y[:, :F] = 0.0  # reference zeroes the first F steps
    return y


if __name__ == "__main__":
    rng = np.random.default_rng(0)
    x = rng.standard_normal((B, N), dtype=np.float32)
    w = (rng.standard_normal(F) * 0.05).astype(np.float32)
    y = kernel(x, w)
    print("kernel ran, y shape:", y.shape)
